# revision 37
# baseline (speedup 1.0000x reference)
"""MLA-style attention (nn_Attention_7868380086611) on 8 TRN2 NeuronCores.

Strategy (v3)
-------------
Head-parallel attention (2 of 16 heads per core).  The query path is fully
absorbed on the host into per-head combined weights (W_dq.T @ W_uq and
W_dq.T @ W_qr.T — weight-only products, same trick as the reference's own
v_eff absorption), so each core computes q/q_r for its 2 heads directly
from the full x with NO collective.  Only the tiny shared kv latent
(c_kv: 512 rows, k_r: 64 rows per token) is computed T-sharded and
AllGathered once (~288 KB per rank); the gather is overlapped with the
q-projection matmuls.  v_eff = W_uv.T @ W_o.T is host-precomputed.

v3 kernel-side improvements over v2:
- PE warm-up matmuls at t=0 so the HAM clock gate opens (2.4 GHz) before
  the real work starts.
- Attention is k-outer with software-pipelined AV matmuls (one k-chunk
  behind the score matmuls) so the tensor queue never stalls on exp; the
  stationary operand (kT/kr/v slice) is reused across the tj blocks of
  one k-chunk, cutting LDWEIGHTS count ~2.5x.
- Causal mask is a multiplicative bf16 0/1 mask applied to exp() output
  (vector 2x mode) instead of a -1e10 f32 add into PSUM (1x mode).
- Softmax denominator accumulates in bf16 (vector 2x) and one
  ones-matmul per (head, tj) on the bf16 accumulator.
- Projection loops are tj-inner so the stationary weight tile is reused
  across 4 matmuls (4x fewer LDWEIGHTS).
"""

import math
import sys

import numpy as np

sys.path.insert(0, "/opt/trn_rl_repo")

import ml_dtypes  # noqa: E402

from concourse import bacc, bass, bass_isa, masks, mybir  # noqa: E402
from concourse.bass_utils import run_bass_kernel_spmd  # noqa: E402
from concourse.tile import TileContext  # noqa: E402

B, T, C = 1, 2048, 2048
NH, HS = 16, 128
NLQ, NLKV, DHR = 1536, 512, 64
NCORES = 8
HPC = NH // NCORES          # heads per core = 2
TS = T // NCORES            # 256-token shard for the kv down-projection
P = 128
LKV = NLKV // P             # 4
CCH = C // P                # 16 c-chunks
TJ = T // 512               # 4 t-chunks of 512
SC = T // P                 # 16 s-chunks
SCALE = 1.0 / math.sqrt(HS + DHR)

BF = mybir.dt.bfloat16
F32 = mybir.dt.float32
Exp = mybir.ActivationFunctionType.Exp
Copy = mybir.ActivationFunctionType.Copy

GKV = NLKV + DHR            # 576 rows in the all-gather buffer


def build_nc():
    nc = bacc.Bacc(None, target_bir_lowering=False, num_devices=NCORES)

    xTp = nc.declare_dram_parameter("xTp", [CCH, P, T], BF, isOutput=False)
    wdkvT = nc.declare_dram_parameter("wdkvT", [1, C, 256], BF, isOutput=False)
    wkrT = nc.declare_dram_parameter("wkrT", [C, DHR], BF, isOutput=False)
    cos2T = nc.declare_dram_parameter("cos2T", [DHR, T], BF, isOutput=False)
    sin2T = nc.declare_dram_parameter("sin2T", [DHR, T], BF, isOutput=False)
    wq = nc.declare_dram_parameter("wq", [CCH, P, HPC * HS], BF, isOutput=False)
    wqr = nc.declare_dram_parameter("wqr", [CCH, P, HPC * DHR], BF, isOutput=False)
    wukT = nc.declare_dram_parameter("wukT", [LKV, P, HPC * HS], BF, isOutput=False)
    bc = nc.declare_dram_parameter("bc", [LKV, P, HPC * HS], BF, isOutput=False)
    xs = nc.declare_dram_parameter("xs", [C, 2 * TS], BF, isOutput=False)
    out = nc.declare_dram_parameter("out", [HPC * T, HS], F32, isOutput=True)

    # phase-1 shard: each core computes HALF the nlkv rows (256) for a
    # 512-token slice (cores 2k/2k+1 share the token slice, different row
    # halves; k_r rows computed redundantly by both) — N=512 matmuls halve
    # the LDWEIGHTS overhead vs the [all-rows x 256-token] sharding.
    GR = NLKV // 2 + DHR        # 320 rows in the all-gather buffer
    TS2 = 2 * TS                # 512-token slice
    cc_in_kv = nc.dram_tensor("cc_in_kv", [GR, TS2], BF)
    cc_out_kv = nc.dram_tensor("cc_out_kv", [NCORES, GR, TS2], BF,
                               addr_space="Shared")

    with TileContext(nc) as tc:
        with (
            tc.tile_pool(name="persist", bufs=1) as persist,
            tc.tile_pool(name="lat", bufs=1) as lat,
            tc.tile_pool(name="proj", bufs=1) as proj,
            tc.tile_pool(name="wts", bufs=1) as wts,
        ):
            # ---- constants ----
            id_bf = persist.tile([P, P], BF)
            masks.make_identity(nc, id_bf[:])
            id_f32 = persist.tile([P, P], F32)
            masks.make_identity(nc, id_f32[:])
            ones_bf = persist.tile([P, 1], BF)
            nc.vector.memset(ones_bf[:], 1.0)
            # 4 multiplicative causal masks [128, 512]: 1 iff t - s - 128*m >= 0
            mask01 = persist.tile([P, 4 * 512], BF)
            nc.vector.memset(mask01[:], 1.0)
            for m in range(4):
                nc.gpsimd.affine_select(
                    out=mask01[:, m * 512:(m + 1) * 512],
                    in_=mask01[:, m * 512:(m + 1) * 512],
                    compare_op=mybir.AluOpType.is_ge,
                    fill=0.0,
                    base=-m * P,
                    channel_multiplier=-1,
                    pattern=[[1, 512]],
                )
            # cos/sin tiles: loaded on the scalar queue but only AFTER the
            # phase-1 bounce stores (emitted below) so they don't delay the
            # AllGather trigger
            cos_sb = persist.tile([DHR, T], BF)
            sin_sb = persist.tile([DHR, T], BF)

            # ---- phase 1: c_kv^T/k_r^T for own T/8 slice -> AllGather.
            # The rank-dependent x column slice comes in pre-sliced (xs) so
            # the SPMD graph stays rank-independent.
            with (
                tc.tile_pool(name="p1w", bufs=1) as p1w,
                tc.tile_pool(name="p1ps", bufs=2, space="PSUM") as p1ps,
                tc.tile_pool(name="p1sh", bufs=3) as p1sh,
                tc.tile_pool(name="p1xs", bufs=1) as p1xs,
            ):
                # PE warm-up: N=512 throwaway matmuls bridging until the xs
                # DMA lands (~9us) so the HAM clock gate opens (2.4 GHz) and
                # STAYS open into the first real accumulation chain.
                junk = p1w.tile([P, 512], BF, name="junk")
                nc.vector.memset(junk[:], 0.0)
                ps_w = p1ps.tile([P, 512], F32, name="ps_warm", tag="warm")
                for _ in range(16):
                    nc.tensor.matmul(ps_w[:], id_bf[:], junk[:],
                                     start=True, stop=True)

                xsl = []
                for cgrp in range(4):
                    tsl = p1xs.tile([P, 4 * TS2], BF, name=f"xsl{cgrp}",
                                    tag=f"xsl{cgrp}")
                    nc.sync.dma_start(
                        tsl[:].rearrange("p (n u) -> p n u", n=4),
                        xs.ap().rearrange("(n p) u -> n p u", p=P)
                        [4 * cgrp:4 * (cgrp + 1)].rearrange("n p u -> p n u"),
                    )
                    xsl.append(tsl)

                def xstile(c):
                    return xsl[c // 4][:, (c % 4) * TS2:(c % 4 + 1) * TS2]

                # wdkv (this core's 256-row half) in 4 group tiles
                w4 = []
                for g in range(4):
                    wt = p1w.tile([P, 4 * 256], BF, name=f"wdkv{g}",
                                  tag=f"wdkv{g}")
                    nc.sync.dma_start(
                        wt[:].rearrange("p (n m) -> p n m", n=4),
                        wdkvT[0].rearrange("(n p) m -> p n m", p=P)
                        [:, 4 * g:4 * (g + 1), :],
                    )
                    w4.append(wt)
                wkr_sb = p1w.tile([P, CCH * DHR], BF, name="wkr_sb")
                nc.sync.dma_start(
                    wkr_sb[:].rearrange("p (n m) -> p n m", n=CCH),
                    wkrT.ap().rearrange("(n p) m -> p n m", p=P),
                )

                def wdkv_sl(c, ls):
                    return w4[c // 4][:, (c % 4) * 256 + ls * P:
                                      (c % 4) * 256 + (ls + 1) * P]

                for ls in range(2):
                    ps = p1ps.tile([P, TS2], F32, name="p1ps_t", tag="p1ps_t")
                    for c in range(CCH):
                        nc.tensor.matmul(
                            ps[:], wdkv_sl(c, ls), xstile(c),
                            start=(c == 0), stop=(c == CCH - 1),
                        )
                    sh = p1sh.tile([P, TS2], BF, name="p1sh_t", tag="p1sh_t")
                    nc.scalar.copy(sh[:], ps[:])
                    nc.scalar.dma_start(
                        cc_in_kv[ls * P:(ls + 1) * P, :], sh[:]
                    )
                ps_kr = p1ps.tile([DHR, TS2], F32, name="ps_kr", tag="p1ps_t")
                for c in range(CCH):
                    nc.tensor.matmul(
                        ps_kr[:],
                        wkr_sb[:, c * DHR:(c + 1) * DHR],
                        xstile(c),
                        start=(c == 0),
                        stop=(c == CCH - 1),
                    )
                sh_kr = p1sh.tile([DHR, TS2], BF, name="sh_kr")
                nc.scalar.copy(sh_kr[:], ps_kr[:])
                nc.scalar.dma_start(cc_in_kv[NLKV // 2:GR, :], sh_kr[:])

                nc.gpsimd.collective_compute(
                    "AllGather",
                    mybir.AluOpType.bypass,
                    replica_groups=[list(range(NCORES))],
                    ins=[cc_in_kv.ap().opt()],
                    outs=[cc_out_kv.ap().opt()],
                )

            nc.scalar.dma_start(cos_sb[:], cos2T[:, :])
            nc.scalar.dma_start(sin_sb[:], sin2T[:, :])

            # ---- projection weights, then full x^T (sync-queue order) ----
            wq_all = wts.tile([P, CCH * HPC * HS], BF)
            nc.sync.dma_start(
                wq_all[:].rearrange("p (n m) -> p n m", n=CCH),
                wq.ap().rearrange("n p m -> p n m"),
            )
            wqr_all = wts.tile([P, CCH * HPC * DHR], BF)
            nc.sync.dma_start(
                wqr_all[:].rearrange("p (n m) -> p n m", n=CCH),
                wqr.ap().rearrange("n p m -> p n m"),
            )
            # x^T chunk loads: 16 simple [128, T] DMAs — cheap descriptor
            # generation vs the rearranged group loads (0.65us vs up to 6us
            # of sync-sequencer time each)
            xt = []
            for cgrp in range(4):
                t = lat.tile([P, 4 * T], BF, name=f"xt{cgrp}", tag=f"xt{cgrp}")
                for j in range(4):
                    nc.sync.dma_start(
                        t[:, j * T:(j + 1) * T], xTp.ap()[4 * cgrp + j]
                    )
                xt.append(t)

            def xtile(c):
                return xt[c // 4][:, (c % 4) * T:(c % 4 + 1) * T]

            wuk_all = wts.tile([P, LKV * HPC * HS], BF)
            nc.sync.dma_start(
                wuk_all[:].rearrange("p (n m) -> p n m", n=LKV),
                wukT.ap().rearrange("n p m -> p n m"),
            )
            b_all = wts.tile([P, LKV * HPC * HS], BF)
            nc.sync.dma_start(
                b_all[:].rearrange("p (n m) -> p n m", n=LKV),
                bc.ap().rearrange("n p m -> p n m"),
            )

            with tc.tile_pool(name="rtmp", bufs=1) as rtmp:

                def rope(dst, src):
                    # dst = src * [cos;cos] + swap_halves(src) * [-sin;sin]
                    sw = rtmp.tile([DHR, T], BF, name="rsw", tag="rsw")
                    nc.sync.dma_start(sw[0:32, :], src[32:64, :])
                    nc.sync.dma_start(sw[32:64, :], src[0:32, :])
                    ta = rtmp.tile([DHR, T], BF, name="rta", tag="rta")
                    tb = rtmp.tile([DHR, T], BF, name="rtb", tag="rtb")
                    nc.vector.tensor_mul(ta[:], src, cos_sb[:])
                    nc.vector.tensor_mul(tb[:], sw[:], sin_sb[:])
                    nc.vector.tensor_add(dst, ta[:], tb[:])

                qT = proj.tile([P, HPC * T], BF)
                kT = proj.tile([P, HPC * T], BF)
                qr_rope = proj.tile([DHR, HPC * T], BF)
                qr2 = proj.tile([P, T], BF)          # merged 2-head qr, pre-split
                qr_h1 = proj.tile([DHR, T], BF)      # head-1 rows on part 0-63
                v_sb = proj.tile([P, SC * HPC * HS], BF)
                kr_rope = proj.tile([DHR, T], BF)

                with tc.tile_pool(name="p5ps", bufs=5, space="PSUM") as p5ps:
                    # q_r^T both heads in one pass (M=128), tj-inner so the
                    # stationary wqr chunk is loaded once per c
                    ps_qr = [
                        p5ps.tile([P, 512], F32, name=f"ps_qr{tj}", tag="p5")
                        for tj in range(TJ)
                    ]
                    for c in range(CCH):
                        for tj in range(TJ):
                            nc.tensor.matmul(
                                ps_qr[tj][:],
                                wqr_all[:, c * HPC * DHR:(c + 1) * HPC * DHR],
                                xtile(c)[:, tj * 512:(tj + 1) * 512],
                                start=(c == 0),
                                stop=(c == CCH - 1),
                            )
                    for tj in range(TJ):
                        nc.vector.tensor_copy(
                            qr2[:, tj * 512:(tj + 1) * 512], ps_qr[tj][:]
                        )
                    nc.sync.dma_start(qr_h1[:, :], qr2[DHR:P, :])
                    rope(qr_rope[:, 0:T], qr2[0:DHR, :])
                    rope(qr_rope[:, T:HPC * T], qr_h1[:, :])

                    # q^T per head, tj-inner
                    for h in range(HPC):
                        ps_q = [
                            p5ps.tile([P, 512], F32, name=f"ps_q{h}_{tj}",
                                      tag="p5")
                            for tj in range(TJ)
                        ]
                        for c in range(CCH):
                            for tj in range(TJ):
                                nc.tensor.matmul(
                                    ps_q[tj][:],
                                    wq_all[:, c * HPC * HS + h * HS:
                                           c * HPC * HS + (h + 1) * HS],
                                    xtile(c)[:, tj * 512:(tj + 1) * 512],
                                    start=(c == 0),
                                    stop=(c == CCH - 1),
                                )
                        for tj in range(TJ):
                            nc.scalar.copy(
                                qT[:, h * T + tj * 512: h * T + (tj + 1) * 512],
                                ps_q[tj][:],
                            )

                    # ---- gathered kv latents (rank r holds nlkv-half r%2 of
                    # token slice r//2; kr lives on the even-rank halves) ----
                    cc_halves = cc_out_kv.ap().rearrange(
                        "(b two) r u -> two b r u", two=2
                    )
                    ckv_t = []
                    for l in range(LKV):
                        t = lat.tile([P, T], BF, name=f"ckv{l}", tag=f"ckv{l}")
                        # split across the sync + scalar queues so the five
                        # gather loads don't serialize on one DMA ring
                        eng = nc.sync if l % 2 == 0 else nc.scalar
                        eng.dma_start(
                            t[:].rearrange("p (g u) -> p g u", g=4),
                            cc_halves[l // 2]
                            [:, (l % 2) * P:(l % 2 + 1) * P, :].rearrange(
                                "g p u -> p g u"
                            ),
                        )
                        ckv_t.append(t)
                    kr_raw = lat.tile([DHR, T], BF)
                    nc.scalar.dma_start(
                        kr_raw[:].rearrange("p (g u) -> p g u", g=4),
                        cc_halves[0][:, NLKV // 2:GR, :].rearrange(
                            "g p u -> p g u"
                        ),
                    )
                    rope(kr_rope[:, :], kr_raw[:, :])

                    # k^T per head, sj-inner (stationary wuk chunk reused)
                    for h in range(HPC):
                        ps_k = [
                            p5ps.tile([P, 512], F32, name=f"ps_k{h}_{sj}",
                                      tag="p5")
                            for sj in range(TJ)
                        ]
                        for l in range(LKV):
                            for sj in range(TJ):
                                nc.tensor.matmul(
                                    ps_k[sj][:],
                                    wuk_all[:, l * HPC * HS + h * HS:
                                            l * HPC * HS + (h + 1) * HS],
                                    ckv_t[l][:, sj * 512:(sj + 1) * 512],
                                    start=(l == 0),
                                    stop=(l == LKV - 1),
                                )
                        for sj in range(TJ):
                            nc.scalar.copy(
                                kT[:, h * T + sj * 512: h * T + (sj + 1) * 512],
                                ps_k[sj][:],
                            )
                    # v~ per s-chunk
                    for sc in range(SC):
                        ps = p5ps.tile([P, HPC * HS], F32, name="ps_v",
                                       tag="p5v", bufs=3)
                        for l in range(LKV):
                            nc.tensor.matmul(
                                ps[:],
                                ckv_t[l][:, sc * P:(sc + 1) * P],
                                b_all[:, l * HPC * HS:(l + 1) * HPC * HS],
                                start=(l == 0),
                                stop=(l == LKV - 1),
                            )
                        nc.vector.tensor_copy(
                            v_sb[:, sc * HPC * HS:(sc + 1) * HPC * HS], ps[:]
                        )

                # ---- attention (causal, k-outer, AV pipelined one k behind).
                with (
                    tc.tile_pool(name="psy", bufs=4, space="PSUM") as psy,
                    tc.tile_pool(name="pss", bufs=4, space="PSUM") as pss,
                    tc.tile_pool(name="atp", bufs=9) as atp,
                    tc.tile_pool(name="accp", bufs=6) as accp,
                    tc.tile_pool(name="spool", bufs=3) as spool,
                    tc.tile_pool(name="opool", bufs=3) as opool,
                ):
                    def vslice(k, h):
                        return v_sb[:, k * HPC * HS + h * HS:
                                    k * HPC * HS + (h + 1) * HS]

                    def tail(h, tj, ps_y, acc):
                        # denominator via gpsimd partition-all-reduce: keeps
                        # the DVE acc-chain dependency OFF the tensor queue
                        # (the ones-matmul used to stall it at every head
                        # tail), and the broadcast result lets us scale yT
                        # elementwise BEFORE the transpose — no per-chunk den
                        # transposes or reciprocals.
                        yT_sb = spool.tile([P, 512], BF, name="yT", tag="yT")
                        nc.scalar.copy(yT_sb[:], ps_y[:])
                        den_bc = spool.tile([P, 512], F32, name="den",
                                            tag="den")
                        nc.gpsimd.partition_all_reduce(
                            den_bc[:], acc[:], channels=P,
                            reduce_op=bass_isa.ReduceOp.add,
                        )
                        rec_bc = spool.tile([P, 512], F32, name="rec",
                                            tag="rec")
                        nc.vector.reciprocal(rec_bc[:], den_bc[:])
                        yT_n = spool.tile([P, 512], BF, name="yTn", tag="yTn")
                        nc.vector.tensor_mul(yT_n[:], yT_sb[:], rec_bc[:])
                        for u in range(4):
                            t0 = tj * 512 + u * P
                            ps_yt = pss.tile([P, P], BF, name="ps_yt",
                                             tag="pss")
                            nc.tensor.transpose(
                                ps_yt[:], yT_n[:, u * P:(u + 1) * P],
                                id_bf[:],
                            )
                            o_sb = opool.tile([P, HS], F32, name="o_sb",
                                              tag="o")
                            nc.scalar.copy(o_sb[:], ps_yt[:])
                            nc.sync.dma_start(
                                out[h * T + t0: h * T + t0 + P, :], o_sb[:]
                            )

                    # snake order: pair dense (low-k) with sparse (high-k)
                    # chunks so tensor density is a uniform ~5 blocks per
                    # iteration — keeps the HAM clock gate at 2.4 GHz
                    snake = []
                    for a, b in zip(range(SC // 2), range(SC - 1, SC // 2 - 1, -1)):
                        snake += [a, b]
                    last_pos = {
                        tj: max(i for i, k in enumerate(snake)
                                if k <= 4 * tj + 3)
                        for tj in range(TJ)
                    }

                    for h in range(HPC):
                        ps_y = {
                            tj: psy.tile([P, 512], F32, name=f"psy{h}_{tj}",
                                         tag="psy")
                            for tj in range(TJ)
                        }
                        acc = {
                            tj: accp.tile([P, 512], BF, name=f"acc{h}_{tj}",
                                          tag="acc")
                            for tj in range(TJ)
                        }
                        pend = {}

                        def emit_av(pos):
                            # AV matmuls for chunk snake[pos] (stationary v
                            # reused); drain (h, tj) when its last AV ran
                            k = snake[pos]
                            for tj, at_prev in pend.pop(pos).items():
                                nc.tensor.matmul(
                                    ps_y[tj][:], vslice(k, h), at_prev[:],
                                    start=(pos == 0),
                                    stop=(pos == last_pos[tj]),
                                )
                            for tj in range(TJ):
                                if pos == last_pos[tj]:
                                    tail(h, tj, ps_y[tj], acc[tj])

                        for pos, k in enumerate(snake):
                            tjs = list(range(k // 4, TJ))
                            ats = {}
                            # sub-groups of <=3 so the 3-deep pss ring can't
                            # deadlock; stationary reused within each group
                            for gi in range(0, len(tjs), 3):
                                grp = tjs[gi:gi + 3]
                                ps_t = {}
                                for tj in grp:
                                    ps_s = pss.tile([P, 512], F32,
                                                    name="ps_s", tag="pss")
                                    nc.tensor.matmul(
                                        ps_s[:],
                                        kT[:, h * T + k * P:
                                           h * T + (k + 1) * P],
                                        qT[:, h * T + tj * 512:
                                           h * T + (tj + 1) * 512],
                                        start=True, stop=False,
                                    )
                                    ps_t[tj] = ps_s
                                for tj in grp:
                                    nc.tensor.matmul(
                                        ps_t[tj][:],
                                        kr_rope[:, k * P:(k + 1) * P],
                                        qr_rope[:, h * T + tj * 512:
                                                h * T + (tj + 1) * 512],
                                        start=False, stop=True,
                                    )
                                for tj in grp:
                                    at = atp.tile([P, 512], BF, name="at",
                                                  tag="at")
                                    nc.scalar.activation(
                                        at[:], ps_t[tj][:], Exp, scale=SCALE
                                    )
                                    if tj == k // 4:
                                        nc.vector.tensor_mul(
                                            at[:], at[:],
                                            mask01[:, (k % 4) * 512:
                                                   (k % 4 + 1) * 512],
                                        )
                                    if pos == 0:
                                        nc.vector.tensor_copy(acc[tj][:],
                                                              at[:])
                                    else:
                                        nc.vector.tensor_add(
                                            acc[tj][:], acc[tj][:], at[:]
                                        )
                                    ats[tj] = at
                            pend[pos] = ats
                            if pos - 1 in pend:
                                emit_av(pos - 1)
                        emit_av(len(snake) - 1)
    nc.finalize()
    return nc


_ROPE_PERM = np.concatenate([np.arange(0, DHR, 2), np.arange(1, DHR, 2)])


def _bf(a):
    return np.ascontiguousarray(a).astype(ml_dtypes.bfloat16)


def _prep_inputs(x, freqs_cos, freqs_sin, W_dq, W_uq, W_dkv, W_uk, W_uv, W_qr,
                 W_kr, W_o):
    """Build the 8 per-core input maps (host-side layout prep, all bf16)."""
    x2 = np.asarray(x, np.float32).reshape(T, C)
    xT = np.ascontiguousarray(x2.T)                  # [C, T]
    xT_bf = _bf(xT).reshape(CCH, P, T)
    WdkvT = np.asarray(W_dkv).T                      # [C, NLKV]
    wkrT = _bf(np.asarray(W_kr)[_ROPE_PERM, :].T)    # [C, DHR], rope-permuted
    cosT = np.asarray(freqs_cos, np.float32).T       # [32, T]
    sinT = np.asarray(freqs_sin, np.float32).T
    cos2T = _bf(np.concatenate([cosT, cosT], axis=0))    # [64, T]
    sin2T = _bf(np.concatenate([-sinT, sinT], axis=0))

    Wdq = np.asarray(W_dq, np.float32)               # [NLQ, C]
    Wuq_mat = np.asarray(W_uq, np.float32).reshape(NLQ, NH * HS)
    Wq_comb = Wdq.T @ Wuq_mat                        # [C, NH*HS]
    Wqr_comb = Wdq.T @ np.asarray(W_qr, np.float32).T    # [C, NH*DHR]
    v_eff = np.asarray(W_uv, np.float32).T @ np.asarray(W_o, np.float32).T
    W_uk_a = np.asarray(W_uk)

    in_maps = []
    for i in range(NCORES):
        h0 = i * HPC
        cols = slice(h0 * HS, (h0 + HPC) * HS)       # 256 output cols
        wqr_cols = np.concatenate(
            [Wqr_comb[:, (h0 + h) * DHR + _ROPE_PERM] for h in range(HPC)],
            axis=1,
        )                                            # [C, HPC*64=128]
        hi = i % 2                                   # nlkv-row half
        tb = i // 2                                  # 512-token block
        in_maps.append({
            "xTp": xT_bf,
            "xs": _bf(xT[:, tb * 512:(tb + 1) * 512]),
            "wdkvT": _bf(np.ascontiguousarray(
                         WdkvT[:, hi * 256:(hi + 1) * 256])
                         .reshape(C, 1, 256).transpose(1, 0, 2)),
            "wkrT": wkrT,
            "cos2T": cos2T,
            "sin2T": sin2T,
            "wq": _bf(Wq_comb[:, cols]).reshape(CCH, P, HPC * HS),
            "wqr": _bf(wqr_cols).reshape(CCH, P, HPC * DHR),
            "wukT": _bf(np.ascontiguousarray(
                        W_uk_a[h0 * HS:(h0 + HPC) * HS, :].T)
                        .reshape(LKV, P, HPC * HS)),
            "bc": _bf(v_eff[:, cols]).reshape(LKV, P, HPC * HS),
        })
    return in_maps


_NC_CACHE = None


def kernel(**inputs):
    global _NC_CACHE
    in_maps = _prep_inputs(**inputs)
    if _NC_CACHE is None:
        _NC_CACHE = build_nc()
    res = run_bass_kernel_spmd(_NC_CACHE, in_maps, core_ids=list(range(NCORES)))
    outs = [np.asarray(res.results[i]["out"], np.float32)
            .reshape(HPC, T, HS).transpose(1, 0, 2).reshape(T, HPC * HS)
            for i in range(NCORES)]
    y = np.concatenate(outs, axis=1).reshape(B, T, C)
    return y


# revision 38
# speedup vs baseline: 1.2385x; 1.2385x over previous
"""MLA-style attention (nn_Attention_7868380086611) on 8 TRN2 NeuronCores.

Strategy (v3)
-------------
Head-parallel attention (2 of 16 heads per core).  The query path is fully
absorbed on the host into per-head combined weights (W_dq.T @ W_uq and
W_dq.T @ W_qr.T — weight-only products, same trick as the reference's own
v_eff absorption), so each core computes q/q_r for its 2 heads directly
from the full x with NO collective.  Only the tiny shared kv latent
(c_kv: 512 rows, k_r: 64 rows per token) is computed T-sharded and
AllGathered once (~288 KB per rank); the gather is overlapped with the
q-projection matmuls.  v_eff = W_uv.T @ W_o.T is host-precomputed.

v3 kernel-side improvements over v2:
- PE warm-up matmuls at t=0 so the HAM clock gate opens (2.4 GHz) before
  the real work starts.
- Attention is k-outer with software-pipelined AV matmuls (one k-chunk
  behind the score matmuls) so the tensor queue never stalls on exp; the
  stationary operand (kT/kr/v slice) is reused across the tj blocks of
  one k-chunk, cutting LDWEIGHTS count ~2.5x.
- Causal mask is a multiplicative bf16 0/1 mask applied to exp() output
  (vector 2x mode) instead of a -1e10 f32 add into PSUM (1x mode).
- Softmax denominator accumulates in bf16 (vector 2x) and one
  ones-matmul per (head, tj) on the bf16 accumulator.
- Projection loops are tj-inner so the stationary weight tile is reused
  across 4 matmuls (4x fewer LDWEIGHTS).
"""

import math
import sys

import numpy as np

sys.path.insert(0, "/opt/trn_rl_repo")

import ml_dtypes  # noqa: E402

from concourse import bacc, bass, masks, mybir  # noqa: E402
from concourse.bass_utils import run_bass_kernel_spmd  # noqa: E402
from concourse.tile import TileContext  # noqa: E402

B, T, C = 1, 2048, 2048
NH, HS = 16, 128
NLQ, NLKV, DHR = 1536, 512, 64
NCORES = 8
HPC = NH // NCORES          # heads per core = 2
TS = T // NCORES            # 256-token shard for the kv down-projection
P = 128
LKV = NLKV // P             # 4
CCH = C // P                # 16 c-chunks
TJ = T // 512               # 4 t-chunks of 512
SC = T // P                 # 16 s-chunks
SCALE = 1.0 / math.sqrt(HS + DHR)

BF = mybir.dt.bfloat16
F32 = mybir.dt.float32
Exp = mybir.ActivationFunctionType.Exp
Copy = mybir.ActivationFunctionType.Copy

GKV = NLKV + DHR            # 576 rows in the all-gather buffer


def build_nc():
    nc = bacc.Bacc(None, target_bir_lowering=False, num_devices=NCORES)

    xTp = nc.declare_dram_parameter("xTp", [CCH, P, T], BF, isOutput=False)
    wdkvT = nc.declare_dram_parameter("wdkvT", [1, C, 256], BF, isOutput=False)
    wkrT = nc.declare_dram_parameter("wkrT", [C, DHR], BF, isOutput=False)
    cos2T = nc.declare_dram_parameter("cos2T", [DHR, T], BF, isOutput=False)
    sin2T = nc.declare_dram_parameter("sin2T", [DHR, T], BF, isOutput=False)
    wq = nc.declare_dram_parameter("wq", [CCH, P, HPC * HS], BF, isOutput=False)
    wqr = nc.declare_dram_parameter("wqr", [CCH, P, HPC * DHR], BF, isOutput=False)
    wukT = nc.declare_dram_parameter("wukT", [LKV, P, HPC * HS], BF, isOutput=False)
    bc = nc.declare_dram_parameter("bc", [LKV, P, HPC * HS], BF, isOutput=False)
    xs = nc.declare_dram_parameter("xs", [C, 2 * TS], BF, isOutput=False)
    out = nc.declare_dram_parameter("out", [HPC * T, HS], F32, isOutput=True)

    # phase-1 shard: each core computes HALF the nlkv rows (256) for a
    # 512-token slice (cores 2k/2k+1 share the token slice, different row
    # halves; k_r rows computed redundantly by both) — N=512 matmuls halve
    # the LDWEIGHTS overhead vs the [all-rows x 256-token] sharding.
    GR = NLKV // 2 + DHR        # 320 rows in the all-gather buffer
    TS2 = 2 * TS                # 512-token slice
    cc_in_kv = nc.dram_tensor("cc_in_kv", [GR, TS2], BF)
    cc_out_kv = nc.dram_tensor("cc_out_kv", [NCORES, GR, TS2], BF,
                               addr_space="Shared")

    with TileContext(nc) as tc:
        with (
            tc.tile_pool(name="persist", bufs=1) as persist,
            tc.tile_pool(name="lat", bufs=1) as lat,
            tc.tile_pool(name="proj", bufs=1) as proj,
            tc.tile_pool(name="wts", bufs=1) as wts,
        ):
            # ---- constants ----
            id_bf = persist.tile([P, P], BF)
            masks.make_identity(nc, id_bf[:])
            id_f32 = persist.tile([P, P], F32)
            masks.make_identity(nc, id_f32[:])
            ones_bf = persist.tile([P, 1], BF)
            nc.vector.memset(ones_bf[:], 1.0)
            # 4 multiplicative causal masks [128, 512]: 1 iff t - s - 128*m >= 0
            mask01 = persist.tile([P, 4 * 512], BF)
            nc.vector.memset(mask01[:], 1.0)
            for m in range(4):
                nc.gpsimd.affine_select(
                    out=mask01[:, m * 512:(m + 1) * 512],
                    in_=mask01[:, m * 512:(m + 1) * 512],
                    compare_op=mybir.AluOpType.is_ge,
                    fill=0.0,
                    base=-m * P,
                    channel_multiplier=-1,
                    pattern=[[1, 512]],
                )
            # cos/sin tiles: loaded on the scalar queue but only AFTER the
            # phase-1 bounce stores (emitted below) so they don't delay the
            # AllGather trigger
            cos_sb = persist.tile([DHR, T], BF)
            sin_sb = persist.tile([DHR, T], BF)

            # ---- phase 1: c_kv^T/k_r^T for own T/8 slice -> AllGather.
            # The rank-dependent x column slice comes in pre-sliced (xs) so
            # the SPMD graph stays rank-independent.
            with (
                tc.tile_pool(name="p1w", bufs=1) as p1w,
                tc.tile_pool(name="p1ps", bufs=2, space="PSUM") as p1ps,
                tc.tile_pool(name="p1sh", bufs=3) as p1sh,
                tc.tile_pool(name="p1xs", bufs=1) as p1xs,
            ):
                # PE warm-up: N=512 throwaway matmuls bridging until the xs
                # DMA lands (~9us) so the HAM clock gate opens (2.4 GHz) and
                # STAYS open into the first real accumulation chain.
                junk = p1w.tile([P, 512], BF, name="junk")
                nc.vector.memset(junk[:], 0.0)
                ps_w = p1ps.tile([P, 512], F32, name="ps_warm", tag="warm")
                for _ in range(16):
                    nc.tensor.matmul(ps_w[:], id_bf[:], junk[:],
                                     start=True, stop=True)

                xsl = []
                for cgrp in range(4):
                    tsl = p1xs.tile([P, 4 * TS2], BF, name=f"xsl{cgrp}",
                                    tag=f"xsl{cgrp}")
                    nc.sync.dma_start(
                        tsl[:].rearrange("p (n u) -> p n u", n=4),
                        xs.ap().rearrange("(n p) u -> n p u", p=P)
                        [4 * cgrp:4 * (cgrp + 1)].rearrange("n p u -> p n u"),
                    )
                    xsl.append(tsl)

                def xstile(c):
                    return xsl[c // 4][:, (c % 4) * TS2:(c % 4 + 1) * TS2]

                # wdkv (this core's 256-row half) in 4 group tiles
                w4 = []
                for g in range(4):
                    wt = p1w.tile([P, 4 * 256], BF, name=f"wdkv{g}",
                                  tag=f"wdkv{g}")
                    nc.sync.dma_start(
                        wt[:].rearrange("p (n m) -> p n m", n=4),
                        wdkvT[0].rearrange("(n p) m -> p n m", p=P)
                        [:, 4 * g:4 * (g + 1), :],
                    )
                    w4.append(wt)
                wkr_sb = p1w.tile([P, CCH * DHR], BF, name="wkr_sb")
                nc.sync.dma_start(
                    wkr_sb[:].rearrange("p (n m) -> p n m", n=CCH),
                    wkrT.ap().rearrange("(n p) m -> p n m", p=P),
                )

                def wdkv_sl(c, ls):
                    return w4[c // 4][:, (c % 4) * 256 + ls * P:
                                      (c % 4) * 256 + (ls + 1) * P]

                for ls in range(2):
                    ps = p1ps.tile([P, TS2], F32, name="p1ps_t", tag="p1ps_t")
                    for c in range(CCH):
                        nc.tensor.matmul(
                            ps[:], wdkv_sl(c, ls), xstile(c),
                            start=(c == 0), stop=(c == CCH - 1),
                        )
                    sh = p1sh.tile([P, TS2], BF, name="p1sh_t", tag="p1sh_t")
                    nc.scalar.copy(sh[:], ps[:])
                    nc.scalar.dma_start(
                        cc_in_kv[ls * P:(ls + 1) * P, :], sh[:]
                    )
                ps_kr = p1ps.tile([DHR, TS2], F32, name="ps_kr", tag="p1ps_t")
                for c in range(CCH):
                    nc.tensor.matmul(
                        ps_kr[:],
                        wkr_sb[:, c * DHR:(c + 1) * DHR],
                        xstile(c),
                        start=(c == 0),
                        stop=(c == CCH - 1),
                    )
                sh_kr = p1sh.tile([DHR, TS2], BF, name="sh_kr")
                nc.scalar.copy(sh_kr[:], ps_kr[:])
                nc.scalar.dma_start(cc_in_kv[NLKV // 2:GR, :], sh_kr[:])

                nc.gpsimd.collective_compute(
                    "AllGather",
                    mybir.AluOpType.bypass,
                    replica_groups=[list(range(NCORES))],
                    ins=[cc_in_kv.ap().opt()],
                    outs=[cc_out_kv.ap().opt()],
                )

            nc.scalar.dma_start(cos_sb[:], cos2T[:, :])
            nc.scalar.dma_start(sin_sb[:], sin2T[:, :])

            # ---- projection weights, then full x^T (sync-queue order) ----
            wq_all = wts.tile([P, CCH * HPC * HS], BF)
            nc.sync.dma_start(
                wq_all[:].rearrange("p (n m) -> p n m", n=CCH),
                wq.ap().rearrange("n p m -> p n m"),
            )
            wqr_all = wts.tile([P, CCH * HPC * DHR], BF)
            nc.sync.dma_start(
                wqr_all[:].rearrange("p (n m) -> p n m", n=CCH),
                wqr.ap().rearrange("n p m -> p n m"),
            )
            # x^T chunk loads: 16 simple [128, T] DMAs — cheap descriptor
            # generation vs the rearranged group loads (0.65us vs up to 6us
            # of sync-sequencer time each)
            xt = []
            for cgrp in range(4):
                t = lat.tile([P, 4 * T], BF, name=f"xt{cgrp}", tag=f"xt{cgrp}")
                for j in range(4):
                    nc.sync.dma_start(
                        t[:, j * T:(j + 1) * T], xTp.ap()[4 * cgrp + j]
                    )
                xt.append(t)

            def xtile(c):
                return xt[c // 4][:, (c % 4) * T:(c % 4 + 1) * T]

            wuk_all = wts.tile([P, LKV * HPC * HS], BF)
            nc.sync.dma_start(
                wuk_all[:].rearrange("p (n m) -> p n m", n=LKV),
                wukT.ap().rearrange("n p m -> p n m"),
            )
            b_all = wts.tile([P, LKV * HPC * HS], BF)
            nc.sync.dma_start(
                b_all[:].rearrange("p (n m) -> p n m", n=LKV),
                bc.ap().rearrange("n p m -> p n m"),
            )

            with tc.tile_pool(name="rtmp", bufs=1) as rtmp:

                def rope(dst, src):
                    # dst = src * [cos;cos] + swap_halves(src) * [-sin;sin]
                    sw = rtmp.tile([DHR, T], BF, name="rsw", tag="rsw")
                    nc.sync.dma_start(sw[0:32, :], src[32:64, :])
                    nc.sync.dma_start(sw[32:64, :], src[0:32, :])
                    ta = rtmp.tile([DHR, T], BF, name="rta", tag="rta")
                    tb = rtmp.tile([DHR, T], BF, name="rtb", tag="rtb")
                    nc.vector.tensor_mul(ta[:], src, cos_sb[:])
                    nc.vector.tensor_mul(tb[:], sw[:], sin_sb[:])
                    nc.vector.tensor_add(dst, ta[:], tb[:])

                qT = proj.tile([P, HPC * T], BF)
                kT = proj.tile([P, HPC * T], BF)
                qr_rope = proj.tile([DHR, HPC * T], BF)
                qr2 = proj.tile([P, T], BF)          # merged 2-head qr, pre-split
                qr_h1 = proj.tile([DHR, T], BF)      # head-1 rows on part 0-63
                v_sb = proj.tile([P, SC * HPC * HS], BF)
                kr_rope = proj.tile([DHR, T], BF)

                with tc.tile_pool(name="p5ps", bufs=5, space="PSUM") as p5ps:
                    # q_r^T both heads in one pass (M=128), tj-inner so the
                    # stationary wqr chunk is loaded once per c
                    ps_qr = [
                        p5ps.tile([P, 512], F32, name=f"ps_qr{tj}", tag="p5")
                        for tj in range(TJ)
                    ]
                    for c in range(CCH):
                        for tj in range(TJ):
                            nc.tensor.matmul(
                                ps_qr[tj][:],
                                wqr_all[:, c * HPC * DHR:(c + 1) * HPC * DHR],
                                xtile(c)[:, tj * 512:(tj + 1) * 512],
                                start=(c == 0),
                                stop=(c == CCH - 1),
                            )
                    for tj in range(TJ):
                        nc.vector.tensor_copy(
                            qr2[:, tj * 512:(tj + 1) * 512], ps_qr[tj][:]
                        )
                    nc.sync.dma_start(qr_h1[:, :], qr2[DHR:P, :])
                    rope(qr_rope[:, 0:T], qr2[0:DHR, :])
                    rope(qr_rope[:, T:HPC * T], qr_h1[:, :])

                    # q^T per head, tj-inner
                    for h in range(HPC):
                        ps_q = [
                            p5ps.tile([P, 512], F32, name=f"ps_q{h}_{tj}",
                                      tag="p5")
                            for tj in range(TJ)
                        ]
                        for c in range(CCH):
                            for tj in range(TJ):
                                nc.tensor.matmul(
                                    ps_q[tj][:],
                                    wq_all[:, c * HPC * HS + h * HS:
                                           c * HPC * HS + (h + 1) * HS],
                                    xtile(c)[:, tj * 512:(tj + 1) * 512],
                                    start=(c == 0),
                                    stop=(c == CCH - 1),
                                )
                        for tj in range(TJ):
                            nc.scalar.copy(
                                qT[:, h * T + tj * 512: h * T + (tj + 1) * 512],
                                ps_q[tj][:],
                            )

                    # ---- gathered kv latents (rank r holds nlkv-half r%2 of
                    # token slice r//2; kr lives on the even-rank halves) ----
                    cc_halves = cc_out_kv.ap().rearrange(
                        "(b two) r u -> two b r u", two=2
                    )
                    ckv_t = []
                    for l in range(LKV):
                        t = lat.tile([P, T], BF, name=f"ckv{l}", tag=f"ckv{l}")
                        nc.sync.dma_start(
                            t[:].rearrange("p (g u) -> p g u", g=4),
                            cc_halves[l // 2]
                            [:, (l % 2) * P:(l % 2 + 1) * P, :].rearrange(
                                "g p u -> p g u"
                            ),
                        )
                        ckv_t.append(t)
                    kr_raw = lat.tile([DHR, T], BF)
                    nc.sync.dma_start(
                        kr_raw[:].rearrange("p (g u) -> p g u", g=4),
                        cc_halves[0][:, NLKV // 2:GR, :].rearrange(
                            "g p u -> p g u"
                        ),
                    )
                    rope(kr_rope[:, :], kr_raw[:, :])

                    # k^T per head, sj-inner (stationary wuk chunk reused)
                    for h in range(HPC):
                        ps_k = [
                            p5ps.tile([P, 512], F32, name=f"ps_k{h}_{sj}",
                                      tag="p5")
                            for sj in range(TJ)
                        ]
                        for l in range(LKV):
                            for sj in range(TJ):
                                nc.tensor.matmul(
                                    ps_k[sj][:],
                                    wuk_all[:, l * HPC * HS + h * HS:
                                            l * HPC * HS + (h + 1) * HS],
                                    ckv_t[l][:, sj * 512:(sj + 1) * 512],
                                    start=(l == 0),
                                    stop=(l == LKV - 1),
                                )
                        for sj in range(TJ):
                            nc.scalar.copy(
                                kT[:, h * T + sj * 512: h * T + (sj + 1) * 512],
                                ps_k[sj][:],
                            )
                    # v~ per s-chunk
                    for sc in range(SC):
                        ps = p5ps.tile([P, HPC * HS], F32, name="ps_v",
                                       tag="p5v", bufs=3)
                        for l in range(LKV):
                            nc.tensor.matmul(
                                ps[:],
                                ckv_t[l][:, sc * P:(sc + 1) * P],
                                b_all[:, l * HPC * HS:(l + 1) * HPC * HS],
                                start=(l == 0),
                                stop=(l == LKV - 1),
                            )
                        nc.vector.tensor_copy(
                            v_sb[:, sc * HPC * HS:(sc + 1) * HPC * HS], ps[:]
                        )

                # ---- attention (causal, k-outer, AV pipelined one k behind).
                with (
                    tc.tile_pool(name="psy", bufs=4, space="PSUM") as psy,
                    tc.tile_pool(name="pss", bufs=4, space="PSUM") as pss,
                    tc.tile_pool(name="atp", bufs=9) as atp,
                    tc.tile_pool(name="accp", bufs=6) as accp,
                    tc.tile_pool(name="spool", bufs=3) as spool,
                    tc.tile_pool(name="opool", bufs=3) as opool,
                ):
                    def vslice(k, h):
                        return v_sb[:, k * HPC * HS + h * HS:
                                    k * HPC * HS + (h + 1) * HS]

                    def tail(h, tj, ps_y, acc):
                        yT_sb = spool.tile([P, 512], BF, name="yT", tag="yT")
                        nc.scalar.copy(yT_sb[:], ps_y[:])
                        ps_d = pss.tile([1, 512], F32, name="ps_d", tag="pss")
                        nc.tensor.matmul(ps_d[:], ones_bf[:], acc[:],
                                         start=True, stop=True)
                        den_sb = spool.tile([1, 512], F32, name="den",
                                            tag="den")
                        nc.scalar.copy(den_sb[:], ps_d[:])
                        for u in range(4):
                            t0 = tj * 512 + u * P
                            ps_dt = pss.tile([P, 1], F32, name="ps_dt",
                                             tag="pss")
                            nc.tensor.transpose(
                                ps_dt[:], den_sb[:, u * P:(u + 1) * P],
                                id_f32[:1, :1],
                            )
                            rec = spool.tile([P, 1], F32, name="rec",
                                             tag="rec")
                            nc.vector.reciprocal(rec[:], ps_dt[:])
                            ps_yt = pss.tile([P, P], BF, name="ps_yt",
                                             tag="pss")
                            nc.tensor.transpose(
                                ps_yt[:], yT_sb[:, u * P:(u + 1) * P],
                                id_bf[:],
                            )
                            o_sb = opool.tile([P, HS], F32, name="o_sb",
                                              tag="o")
                            nc.scalar.activation(
                                o_sb[:], ps_yt[:], Copy, scale=rec[:]
                            )
                            nc.sync.dma_start(
                                out[h * T + t0: h * T + t0 + P, :], o_sb[:]
                            )

                    # snake order: pair dense (low-k) with sparse (high-k)
                    # chunks so tensor density is a uniform ~5 blocks per
                    # iteration — keeps the HAM clock gate at 2.4 GHz
                    snake = []
                    for a, b in zip(range(SC // 2), range(SC - 1, SC // 2 - 1, -1)):
                        snake += [a, b]
                    last_pos = {
                        tj: max(i for i, k in enumerate(snake)
                                if k <= 4 * tj + 3)
                        for tj in range(TJ)
                    }

                    for h in range(HPC):
                        ps_y = {
                            tj: psy.tile([P, 512], F32, name=f"psy{h}_{tj}",
                                         tag="psy")
                            for tj in range(TJ)
                        }
                        acc = {
                            tj: accp.tile([P, 512], BF, name=f"acc{h}_{tj}",
                                          tag="acc")
                            for tj in range(TJ)
                        }
                        pend = {}

                        def emit_av(pos):
                            # AV matmuls for chunk snake[pos] (stationary v
                            # reused); drain (h, tj) when its last AV ran
                            k = snake[pos]
                            for tj, at_prev in pend.pop(pos).items():
                                nc.tensor.matmul(
                                    ps_y[tj][:], vslice(k, h), at_prev[:],
                                    start=(pos == 0),
                                    stop=(pos == last_pos[tj]),
                                )
                            for tj in range(TJ):
                                if pos == last_pos[tj]:
                                    tail(h, tj, ps_y[tj], acc[tj])

                        for pos, k in enumerate(snake):
                            tjs = list(range(k // 4, TJ))
                            ats = {}
                            # sub-groups of <=3 so the 3-deep pss ring can't
                            # deadlock; stationary reused within each group
                            for gi in range(0, len(tjs), 3):
                                grp = tjs[gi:gi + 3]
                                ps_t = {}
                                for tj in grp:
                                    ps_s = pss.tile([P, 512], F32,
                                                    name="ps_s", tag="pss")
                                    nc.tensor.matmul(
                                        ps_s[:],
                                        kT[:, h * T + k * P:
                                           h * T + (k + 1) * P],
                                        qT[:, h * T + tj * 512:
                                           h * T + (tj + 1) * 512],
                                        start=True, stop=False,
                                    )
                                    ps_t[tj] = ps_s
                                for tj in grp:
                                    nc.tensor.matmul(
                                        ps_t[tj][:],
                                        kr_rope[:, k * P:(k + 1) * P],
                                        qr_rope[:, h * T + tj * 512:
                                                h * T + (tj + 1) * 512],
                                        start=False, stop=True,
                                    )
                                for tj in grp:
                                    at = atp.tile([P, 512], BF, name="at",
                                                  tag="at")
                                    nc.scalar.activation(
                                        at[:], ps_t[tj][:], Exp, scale=SCALE
                                    )
                                    if tj == k // 4:
                                        nc.vector.tensor_mul(
                                            at[:], at[:],
                                            mask01[:, (k % 4) * 512:
                                                   (k % 4 + 1) * 512],
                                        )
                                    if pos == 0:
                                        nc.vector.tensor_copy(acc[tj][:],
                                                              at[:])
                                    else:
                                        nc.vector.tensor_add(
                                            acc[tj][:], acc[tj][:], at[:]
                                        )
                                    ats[tj] = at
                            pend[pos] = ats
                            if pos - 1 in pend:
                                emit_av(pos - 1)
                        emit_av(len(snake) - 1)
    nc.finalize()
    return nc


_ROPE_PERM = np.concatenate([np.arange(0, DHR, 2), np.arange(1, DHR, 2)])


def _bf(a):
    return np.ascontiguousarray(a).astype(ml_dtypes.bfloat16)


def _prep_inputs(x, freqs_cos, freqs_sin, W_dq, W_uq, W_dkv, W_uk, W_uv, W_qr,
                 W_kr, W_o):
    """Build the 8 per-core input maps (host-side layout prep, all bf16)."""
    x2 = np.asarray(x, np.float32).reshape(T, C)
    xT = np.ascontiguousarray(x2.T)                  # [C, T]
    xT_bf = _bf(xT).reshape(CCH, P, T)
    WdkvT = np.asarray(W_dkv).T                      # [C, NLKV]
    wkrT = _bf(np.asarray(W_kr)[_ROPE_PERM, :].T)    # [C, DHR], rope-permuted
    cosT = np.asarray(freqs_cos, np.float32).T       # [32, T]
    sinT = np.asarray(freqs_sin, np.float32).T
    cos2T = _bf(np.concatenate([cosT, cosT], axis=0))    # [64, T]
    sin2T = _bf(np.concatenate([-sinT, sinT], axis=0))

    Wdq = np.asarray(W_dq, np.float32)               # [NLQ, C]
    Wuq_mat = np.asarray(W_uq, np.float32).reshape(NLQ, NH * HS)
    Wq_comb = Wdq.T @ Wuq_mat                        # [C, NH*HS]
    Wqr_comb = Wdq.T @ np.asarray(W_qr, np.float32).T    # [C, NH*DHR]
    v_eff = np.asarray(W_uv, np.float32).T @ np.asarray(W_o, np.float32).T
    W_uk_a = np.asarray(W_uk)

    in_maps = []
    for i in range(NCORES):
        h0 = i * HPC
        cols = slice(h0 * HS, (h0 + HPC) * HS)       # 256 output cols
        wqr_cols = np.concatenate(
            [Wqr_comb[:, (h0 + h) * DHR + _ROPE_PERM] for h in range(HPC)],
            axis=1,
        )                                            # [C, HPC*64=128]
        hi = i % 2                                   # nlkv-row half
        tb = i // 2                                  # 512-token block
        in_maps.append({
            "xTp": xT_bf,
            "xs": _bf(xT[:, tb * 512:(tb + 1) * 512]),
            "wdkvT": _bf(np.ascontiguousarray(
                         WdkvT[:, hi * 256:(hi + 1) * 256])
                         .reshape(C, 1, 256).transpose(1, 0, 2)),
            "wkrT": wkrT,
            "cos2T": cos2T,
            "sin2T": sin2T,
            "wq": _bf(Wq_comb[:, cols]).reshape(CCH, P, HPC * HS),
            "wqr": _bf(wqr_cols).reshape(CCH, P, HPC * DHR),
            "wukT": _bf(np.ascontiguousarray(
                        W_uk_a[h0 * HS:(h0 + HPC) * HS, :].T)
                        .reshape(LKV, P, HPC * HS)),
            "bc": _bf(v_eff[:, cols]).reshape(LKV, P, HPC * HS),
        })
    return in_maps


_NC_CACHE = None


def kernel(**inputs):
    global _NC_CACHE
    in_maps = _prep_inputs(**inputs)
    if _NC_CACHE is None:
        _NC_CACHE = build_nc()
    res = run_bass_kernel_spmd(_NC_CACHE, in_maps, core_ids=list(range(NCORES)))
    outs = [np.asarray(res.results[i]["out"], np.float32)
            .reshape(HPC, T, HS).transpose(1, 0, 2).reshape(T, HPC * HS)
            for i in range(NCORES)]
    y = np.concatenate(outs, axis=1).reshape(B, T, C)
    return y
